# revision 6
# baseline (speedup 1.0000x reference)
"""Trainium2 Bass kernel for nn_InterpolantActivation (histogram_binning).

y[b, j] = interp1d(grid, act_array[seg(j)], x[b, j]) + c_seg(j)
  where grid = linspace(-5, 5, 50), seg(j) = j // 1024, and c_s is the
  constant from the reference's masked formulation (other activations
  evaluated at x = 0).

The 49-segment piecewise-linear interpolant is evaluated exactly as an
affine base plus a 48-term relu series in u = 4.9*x + 24.5 (unit knot
spacing, knots at integers 1..48), split two-sided around the anchor
bin 24 so partial sums stay small:

    y = A*u + B + sum_{k=25..48} d_k*relu(u - k)
               + sum_{k=1..24}  d_k*relu(k - u)

All table-derived constants (A, B, d_k per column segment) are folded
on the host from act_array.  On device, per [128, 1024] tile:
  - ScalarE (ACT) generates each unscaled term Relu(+-4.9*x + bias)
    straight from x (free scale/bias of the ACTIVATE instruction),
  - VectorE folds it in with one stock scalar_tensor_tensor:
    acc = (t * d_k) + acc,
so the two engines stream in parallel.  Raw Block + manual semaphores
(double/triple buffered DMA in, term ring, DMA out).

Pure data parallel across 8 NeuronCores: rows sharded 8192 -> 8 x 1024.
"""

import os
import sys
from contextlib import ExitStack

import numpy as np

for _p in ("/opt/trn_rl_repo", "/root/.axon_site/_ro/trn_rl_repo"):
    if _p not in sys.path:
        sys.path.insert(0, _p)

B_FULL, L = 8192, 4096
N_CORES = 8
B_SHARD = B_FULL // N_CORES  # 1024
N_ACT, G = 4, 50
SPLIT = L // N_ACT  # 1024
TILE_P, TILE_F = 128, 1024
NB = 3   # x/acc buffer slots
NR = 8   # ACT term-tile ring slots
NTERM = 48
ANCHOR = 24

LAST_EXEC_NS = None
_CACHE = {}


def _consts(act_array):
    """Host-folded constants (float64)."""
    act = np.asarray(act_array, dtype=np.float64)
    xg = np.linspace(-5.0, 5.0, G)

    def interp0(yg):
        ind = int(np.clip(np.searchsorted(xg, 0.0) - 1, 0, G - 2))
        sl = (yg[ind + 1] - yg[ind]) / (xg[ind + 1] - xg[ind])
        return yg[ind] + sl * (0.0 - xg[ind])

    v0 = np.array([interp0(act[i]) for i in range(N_ACT)])
    c_seg = v0.sum() - v0

    sl = act[:, 1:] - act[:, :-1]            # [4, 49] u-space slopes
    d = sl[:, 1:] - sl[:, :-1]               # [4, 48]; d[:, k-1] is d_k
    A = sl[:, ANCHOR]                        # slope on bin [24, 25]
    Bc = act[:, ANCHOR] - ANCHOR * A + c_seg  # y(u=24) - 24*A + c_s
    # x-space affine base: u = 4.9*x + 24.5 -> A*u + B = (4.9*A)*x + (24.5*A + B)
    Ax = 4.9 * A
    Bx = 24.5 * A + Bc
    return Ax, Bx, d


def _build(Ax, Bx, d):
    import concourse.bass as bass
    import concourse.mybir as mybir

    f32 = mybir.dt.float32
    add, mult = mybir.AluOpType.add, mybir.AluOpType.mult
    Relu = mybir.ActivationFunctionType.Relu

    # term list: (scale, bias, is_right, k) — ACT computes Relu(scale*x+bias)
    # right (k=25..48): relu(u-k) = Relu(4.9x + 24.5-k)
    # left  (k=1..24):  relu(k-u) = Relu(-4.9x + k-24.5)
    terms = []
    for k in range(ANCHOR + 1, NTERM + 1):     # 25..48
        terms.append((4.9, 24.5 - k, k))
    for k in range(1, ANCHOR + 1):             # 1..24
        terms.append((-4.9, k - 24.5, k))
    assert len(terms) == NTERM

    nc = bass.Bass(trn_type="TRN2")
    x = nc.dram_tensor("x", [B_SHARD, L], f32, kind="ExternalInput")
    biases = nc.dram_tensor("biases", [TILE_P, NTERM], f32, kind="ExternalInput")
    out = nc.dram_tensor("out", [B_SHARD, L], f32, kind="ExternalOutput")

    n_tiles = (B_SHARD // TILE_P) * N_ACT  # 32

    def tile_slice(i):
        r, s = divmod(i, N_ACT)
        rs, cs = r * TILE_P, s * SPLIT
        return s, (slice(rs, rs + TILE_P), slice(cs, cs + TILE_F))

    with ExitStack() as ctx:
        xts = [ctx.enter_context(nc.sbuf_tensor(f"xt{i}", [TILE_P, TILE_F], f32))
               for i in range(NB)]
        ats = [ctx.enter_context(nc.sbuf_tensor(f"at{i}", [TILE_P, TILE_F], f32))
               for i in range(NB)]
        tts = [ctx.enter_context(nc.sbuf_tensor(f"tt{i}", [TILE_P, TILE_F], f32))
               for i in range(NR)]
        bias_t = ctx.enter_context(nc.sbuf_tensor("bias_t", [TILE_P, NTERM], f32))
        s_bias = ctx.enter_context(nc.semaphore())
        s_in = ctx.enter_context(nc.semaphore())
        s_act = ctx.enter_context(nc.semaphore())
        s_stt = ctx.enter_context(nc.semaphore())
        s_out = ctx.enter_context(nc.semaphore())
        blk = ctx.enter_context(nc.Block())

        @blk.sync
        def _(sync):
            sync.dma_start(bias_t[:], biases[:]).then_inc(s_bias, 16)
            for i in range(n_tiles):
                slot = i % NB
                _, sl = tile_slice(i)
                if i >= NB:
                    # x slot free once ACT finished the prior occupant's
                    # terms AND DVE ran its affine init (first STT of that
                    # tile implies the init, which reads x, already ran).
                    sync.wait_ge(s_act, NTERM * (i - NB + 1))
                    sync.wait_ge(s_stt, NTERM * (i - NB) + 1)
                sync.dma_start(xts[slot][:], x[sl[0], sl[1]]).then_inc(s_in, 16)

        @blk.scalar
        def _(scalar):
            g = 0
            scalar.wait_ge(s_bias, 16)
            for i in range(n_tiles):
                slot = i % NB
                scalar.wait_ge(s_in, 16 * (i + 1))
                for j, (sc, bias, _k) in enumerate(terms):
                    if g >= NR:
                        scalar.wait_ge(s_stt, g - NR + 1)
                    nc.scalar.activation(
                        tts[g % NR][:], xts[slot][:], Relu,
                        bias=bias_t[:, j:j + 1], scale=float(sc),
                    ).then_inc(s_act, 1)
                    g += 1

        @blk.vector
        def _(vector):
            g = 0
            for i in range(n_tiles):
                slot = i % NB
                seg, _sl = tile_slice(i)
                vector.wait_ge(s_in, 16 * (i + 1))
                if i >= NB:
                    vector.wait_ge(s_out, 16 * (i - NB + 1))
                nc.vector.tensor_scalar(
                    ats[slot][:], xts[slot][:],
                    float(Ax[seg]), float(Bx[seg]), mult, add,
                )
                for (_sc, _bias, k) in terms:
                    vector.wait_ge(s_act, g + 1)
                    nc.vector.scalar_tensor_tensor(
                        ats[slot][:], tts[g % NR][:], float(d[seg, k - 1]),
                        ats[slot][:], mult, add,
                    ).then_inc(s_stt, 1)
                    g += 1

        @blk.gpsimd
        def _(gpsimd):
            for i in range(n_tiles):
                slot = i % NB
                _, sl = tile_slice(i)
                gpsimd.wait_ge(s_stt, NTERM * (i + 1))
                gpsimd.dma_start(out[sl[0], sl[1]], ats[slot][:]).then_inc(
                    s_out, 16
                )

    return nc


def kernel(x, act_array):
    global LAST_EXEC_NS
    from concourse.bass_utils import run_bass_kernel_spmd

    x = np.ascontiguousarray(np.asarray(x, dtype=np.float32))
    assert x.shape == (B_FULL, L), x.shape

    key = np.asarray(act_array, dtype=np.float32).tobytes()
    if key not in _CACHE:
        Ax, Bx, d = _consts(act_array)
        _CACHE[key] = _build(Ax, Bx, d)
    nc = _CACHE[key]

    terms_bias = ([24.5 - k for k in range(ANCHOR + 1, NTERM + 1)]
                  + [k - 24.5 for k in range(1, ANCHOR + 1)])
    bias_np = np.tile(np.asarray(terms_bias, dtype=np.float32), (TILE_P, 1))
    bias_np = np.ascontiguousarray(bias_np)
    shards = x.reshape(N_CORES, B_SHARD, L)
    in_maps = [{"x": shards[i], "biases": bias_np} for i in range(N_CORES)]
    want_trace = bool(int(os.environ.get("K_TRACE", "0")))
    try:
        res = run_bass_kernel_spmd(
            nc, in_maps, core_ids=list(range(N_CORES)), trace=want_trace,
        )
    except ModuleNotFoundError:
        # NTFF profiling hook unavailable in this environment
        res = run_bass_kernel_spmd(
            nc, in_maps, core_ids=list(range(N_CORES)), trace=False,
        )
    LAST_EXEC_NS = res.exec_time_ns
    out = np.concatenate([r["out"] for r in res.results], axis=0)
    return out.astype(np.float32)
